# revision 21
# baseline (speedup 1.0000x reference)
"""Ewald reciprocal-space kernel for Trainium2 (8 NeuronCores, SPMD) — v3.

Math (per batch b):
    s        = cell_inv @ x          (fractional coords)
    theta    = 2*pi * (kvec . s)     (B, N, NK) phases
    S_re/S_im= sum_n q_n {cos,sin}(theta)          (structure factor)
    recip_n  = sum_k expfac_k (S_re cos + S_im sin)
    phi      = recip * BOHR/(pi*V) - q * 2*bewald*BOHR/sqrt(pi)
    returns (0.5*q*phi, phi)

Sharding: 8 cores = 2 batches x 4 k-shards (1024 k-vectors each). Each core
computes its full-N, shard-K contribution to recip; host sums the 4 shard
partials per batch and applies the final affine.

Device pipeline per core (N=4096 as 32 chunks of 128 partitions):
  C = round(u) - u in PSUM via a 3-pass magic chain: mm1 = u + M in one
      K=4 matmul (ones row x MAGIC row; fp32 rounding pins M + round(u)),
      mm2 = -M (rank-1), mm3 = -u.  (Validated bit-accurate on the device
      path; fp8 DoubleRow for mm2 diverges on device — not used.)
  sin half: ACT Sin reads C from PSUM directly (scale ~ -2pi), fp16 out.
  cos half: custom-DVE even deg-6 polynomial COS6 (q(0)=1 constrained,
      ~1.8e-3 abs err) — sign-free via evenness, no re-wrap needed, and
      it frees the ACT engine of the second Sin pass.
  Sm: flipped N=1 matmuls accumulated across all 32 chunks in one PSUM
      chain (single start, skip_group_check).
  cs transposed for pass 2: 26 chunks via the SP-queue xbar DMA; 6 chunks
      via PE transpose instructions (fp16 psum) + ACT/DVE copy-out, which
      offloads the single legal DMA-transpose queue (ACT-queue xbar
      transposes corrupt data on this device).
  recip: flipped matmuls, 16-deep psum chains per n-chunk -> [128, 32].
  Input loads / output stores ride the gpsimd (Pool) SWDGE queue.
"""

import math
from contextlib import ExitStack

import numpy as np

BOHR = 1.8897261258369282

B, N, NK = 2, 4096, 4096
NCORES = 8
KSH = NK // 4          # k-vectors per core
NCH = N // 128         # 32 n-chunks
NSL = 2 * KSH // 128   # 16 slabs: 0-7 sin(k=128s+p), 8-15 cos

MAGIC = 12582912.0              # 1.5 * 2**23 fp32 round-to-nearest magic
NEG2PI_SAFE = -2.0 * math.pi * (1.0 - 2e-5)   # |scale*C| <= pi with margin

# cos(2*pi*C) ~= ((c3*x + c2)*x + c1)*x + 1, x = C^2, |C| <= 0.505
COS_C = (-60.15944654, 61.41036513, -19.59589621)

PE_TP_DVE = frozenset((4, 14, 24, 31))  # PE-transposed chunks, DVE copy-out
PE_TP_ACT = frozenset((9, 19, 29))      # PE-transposed chunks, ACT copy-out

_PROG = {}
_OPS = {}


def _register_dve_ops():
    """Register the COS6 polynomial op in the custom-DVE registry."""
    if _OPS:
        return _OPS
    from concourse import dve_ops
    from concourse.dve_spec import Spec, Src0, C0, C1, C2, One
    from concourse.dve_spec import lower as dve_lower, _has_src1 as has_src1
    from concourse.dve_ops import DveOp
    from concourse.dve_uop import DveOpSpec

    def make(name, spec):
        for op in dve_ops.OPS:
            if op.name == name:
                return op
        shas = {}
        for ver in ("v3", "v4"):
            s = DveOpSpec(name=name, opcode=1, uops=dve_lower(spec, ver=ver),
                          rd1_en=has_src1(spec))
            shas[ver] = s.sha(ver)
        op = DveOp(name, spec, subdim=False, uops_sha=shas)
        dve_ops.OPS.append(op)
        dve_ops.CUSTOM_DVE_SPECS[name] = op.spec
        dve_ops._SUB_OPCODE_FOR_NAME[name] = (
            dve_ops._CUSTOM_DVE_ROW_BASE + len(dve_ops.OPS) - 1)
        assert dve_ops._SUB_OPCODE_FOR_NAME[name] < 0x20
        return op

    _x = Src0 * Src0

    def _cos6_ref(in0, in1, c0, c1, c2):
        x = in0.astype(np.float32) ** 2
        return ((c0 * x + c1) * x + c2) * x + np.float32(1.0)

    _OPS["COS6"] = make(
        "COS6_EWALD",
        Spec(body=((C0 * _x + C1) * _x + C2) * _x + One, reference=_cos6_ref))
    return _OPS


def _build_program():
    import concourse.bass as bass
    import concourse.bacc as bacc
    import concourse.tile as tile
    import concourse.mybir as mybir

    ops = _register_dve_ops()
    COS6 = ops["COS6"]

    F32 = mybir.dt.float32
    F32R = mybir.dt.float32r
    F16 = mybir.dt.float16

    nc = bacc.Bacc(trn_type="TRN2", target_bir_lowering=False, debug=False)

    cts4_d = nc.dram_tensor("cts4", [4, N], F32, kind="ExternalInput").ap()
    km4_d = nc.dram_tensor("km4", [4, KSH], F32, kind="ExternalInput").ap()
    kmn_d = nc.dram_tensor("kmn", [3, KSH], F32, kind="ExternalInput").ap()
    qT_d = nc.dram_tensor("qT", [128, NCH], F32, kind="ExternalInput").ap()
    ef2_d = nc.dram_tensor("ef2", [128, NSL], F32, kind="ExternalInput").ap()
    id_d = nc.dram_tensor("ident", [128, 128], F16, kind="ExternalInput").ap()
    recp_d = nc.dram_tensor("recp", [128, NCH], F32, kind="ExternalOutput").ap()

    with tile.TileContext(nc) as tc, ExitStack() as ctx:
        const = ctx.enter_context(tc.tile_pool(name="const", bufs=1))
        pu = ctx.enter_context(tc.tile_pool(name="pu", bufs=3, space="PSUM"))
        psm = ctx.enter_context(tc.tile_pool(name="psm", bufs=1, space="PSUM"))
        pt = ctx.enter_context(tc.tile_pool(name="pt", bufs=1, space="PSUM"))
        wk = ctx.enter_context(tc.tile_pool(name="wk", bufs=8))

        # ---- loads on the gpsimd (Pool) SWDGE queue ----
        # order: mm1 deps (km4, cts head), mm3 dep (kmn), then the tiny
        # tiles needed by the first emit_back (qT for Sm flips), then the
        # bulk of the coords.
        km4 = const.tile([4, KSH], F32R)
        nc.gpsimd.dma_start(out=km4[:, 0:512], in_=km4_d[:, 0:512].bitcast(F32R))
        cts4 = const.tile([4, N], F32R)
        nc.gpsimd.dma_start(out=cts4[:, 0:256],
                            in_=cts4_d[:, 0:256].bitcast(F32R))
        kmn = const.tile([3, KSH], F32R)
        nc.gpsimd.dma_start(out=kmn[:, 0:512], in_=kmn_d[:, 0:512].bitcast(F32R))
        qt = const.tile([128, NCH], F32)
        nc.gpsimd.dma_start(out=qt[:, :], in_=qT_d)
        nc.gpsimd.dma_start(out=km4[:, 512:KSH],
                            in_=km4_d[:, 512:KSH].bitcast(F32R))
        nc.gpsimd.dma_start(out=kmn[:, 512:KSH],
                            in_=kmn_d[:, 512:KSH].bitcast(F32R))
        ef = const.tile([128, NSL], F32)
        nc.gpsimd.dma_start(out=ef[:, :], in_=ef2_d)
        ident = const.tile([128, 128], F16)
        nc.gpsimd.dma_start(out=ident[:, :], in_=id_d)
        nc.gpsimd.dma_start(out=cts4[:, 256:1024],
                            in_=cts4_d[:, 256:1024].bitcast(F32R))
        nc.gpsimd.dma_start(out=cts4[:, 1024:2560],
                            in_=cts4_d[:, 1024:2560].bitcast(F32R))
        nc.gpsimd.dma_start(out=cts4[:, 2560:N],
                            in_=cts4_d[:, 2560:N].bitcast(F32R))

        qt16 = const.tile([128, NCH], F16)
        nc.scalar.activation(qt16[:, :], qt[:, :],
                             mybir.ActivationFunctionType.Copy,
                             bias=0.0, scale=1.0)
        ones = const.tile([1, 128], F32R)
        nc.vector.memset(ones.bitcast(F32)[:, :], 1.0)
        mrow = const.tile([1, 512], F32R)
        nc.vector.memset(mrow.bitcast(F32)[:, :], MAGIC)
        mrow_n = const.tile([1, 512], F32R)
        nc.vector.memset(mrow_n.bitcast(F32)[:, :], -MAGIC)

        s_warm = const.tile([128, 1], F32)
        nc.vector.memset(s_warm[:, :], 0.0)
        sin_warm = const.tile([128, 1], F16)
        nc.scalar.activation(sin_warm[:, :], s_warm[:, :],
                             mybir.ActivationFunctionType.Sin,
                             bias=0.0, scale=NEG2PI_SAFE)

        # warm the PE p-state during input loads
        warm = pu.tile([128, KSH], F32, tag="C", name="warm")
        for _ in range(6):
            nc.tensor.matmul(warm[0:1, 0:512], lhsT=ones[:, 0:1],
                             rhs=mrow[:, :], start=True, stop=True)

        csT = const.tile([128, NSL, N], F16)
        smp = psm.tile([128, NSL], F32, name="smp")

        def emit_front(t, cs):
            """magic chain + trig for chunk t into cs."""
            C = pu.tile([128, KSH], F32, tag="C", name=f"C{t}")
            for off in range(0, KSH, 512):
                co = C[:, off:off + 512]
                nc.tensor.matmul(co, lhsT=cts4[:, 128 * t:128 * (t + 1)],
                                 rhs=km4[:, off:off + 512],
                                 start=True, stop=False)
                nc.tensor.matmul(co, lhsT=ones[:, :], rhs=mrow_n[:, :],
                                 start=False, stop=False)
                nc.tensor.matmul(co, lhsT=cts4[0:3, 128 * t:128 * (t + 1)],
                                 rhs=kmn[:, off:off + 512],
                                 start=False, stop=True)
            nc.scalar.activation(cs[:, 0:KSH], C[:, :],
                                 mybir.ActivationFunctionType.Sin,
                                 bias=0.0, scale=NEG2PI_SAFE)
            nc.vector._custom_dve(COS6, out=cs[:, KSH:2 * KSH], in0=C[:, :],
                                  s0=COS_C[0], s1=COS_C[1], imm2=COS_C[2])

        def emit_pe_tp_half(t, cs, half, copy_eng):
            """PE-transpose 8 slabs of one cs half into psum, copy to csT."""
            tp = pt.tile([128, KSH], F16, tag="tp", name=f"tp{t}_{half}")
            base = half * 8
            for j in range(8):
                nc.tensor.transpose(tp[:, 128 * j:128 * (j + 1)],
                                    cs[:, 128 * (base + j):128 * (base + j + 1)],
                                    ident[:, :])
            dst = csT[:, base:base + 8, 128 * t:128 * (t + 1)]
            if copy_eng == "dve":
                nc.vector.tensor_copy(dst, tp[:, :])
            else:
                nc.scalar.activation(dst, tp[:, :],
                                     mybir.ActivationFunctionType.Copy,
                                     bias=0.0, scale=1.0)

        def emit_tp(t, cs):
            """transpose for a chunk whose cs is complete."""
            pe_tp = t in PE_TP_DVE or t in PE_TP_ACT
            if pe_tp:
                eng = "dve" if t in PE_TP_DVE else "act"
                emit_pe_tp_half(t, cs, 0, eng)
                emit_pe_tp_half(t, cs, 1, eng)
            else:
                nc.sync.dma_start_transpose(
                    out=csT[:, :, 128 * t:128 * (t + 1)], in_=cs)

        def emit_flips(t, cs):
            for s in range(NSL):
                nc.tensor.matmul(smp[:, s:s + 1],
                                 lhsT=cs[:, 128 * s:128 * (s + 1)],
                                 rhs=qt16[:, t:t + 1],
                                 start=(t == 0 and s == 0),
                                 stop=(t == NCH - 1 and s == NSL - 1),
                                 skip_group_check=True)

        prev = None
        for t in range(NCH):
            cs = wk.tile([128, 2 * KSH], F16, tag="cs", name=f"cs{t}")
            if prev is not None:
                emit_tp(prev[0], prev[1])
            emit_front(t, cs)
            if prev is not None:
                emit_flips(prev[0], prev[1])
            prev = (t, cs)
        emit_tp(prev[0], prev[1])
        emit_flips(prev[0], prev[1])

        # ---- w = expfac * S (fp16 columns) ----
        w = const.tile([128, NSL], F16)
        nc.vector.tensor_tensor(out=w[:, :], in0=smp[:, :], in1=ef[:, :],
                                op=mybir.AluOpType.mult)

        # ---- pass 2: recip via 16-deep flipped-matmul psum chains ----
        r_acc = pt.tile([128, NCH], F32, tag="tp", name="r_acc")
        outsb = const.tile([128, NCH], F32)
        piece = 0
        for c in range(NCH):
            for s in range(NSL):
                nc.tensor.matmul(r_acc[:, c:c + 1],
                                 lhsT=csT[:, s, 128 * c:128 * (c + 1)],
                                 rhs=w[:, s:s + 1],
                                 start=(s == 0), stop=(s == NSL - 1))
            if c in (7, 15, 23):
                lo, hi = piece * 8, (piece + 1) * 8
                nc.vector.tensor_copy(outsb[:, lo:hi], r_acc[:, lo:hi])
                nc.gpsimd.dma_start(out=recp_d[:, lo:hi], in_=outsb[:, lo:hi])
                piece += 1
        nc.vector.tensor_copy(outsb[:, 24:NCH], r_acc[:, 24:NCH])
        nc.sync.dma_start(out=recp_d[:, 24:NCH], in_=outsb[:, 24:NCH])

    nc.compile()
    return nc


def _get_prog():
    if "prog" not in _PROG:
        _PROG["prog"] = _build_program()
    return _PROG["prog"]


def _make_in_maps(coords, q, cell_inv, kvec, expfac):
    in_maps = []
    ident = np.eye(128, dtype=np.float16)
    for c in range(NCORES):
        b, ks = divmod(c, NCORES // B)
        sl = slice(KSH * ks, KSH * (ks + 1))
        ct = np.ascontiguousarray(coords[b].T, dtype=np.float32)
        cts4 = np.concatenate([ct, np.ones((1, N), np.float32)], axis=0)
        ef = np.ascontiguousarray(expfac[sl], dtype=np.float32)
        ef8 = ef.reshape(NSL // 2, 128).T          # [128, 8]
        kmT = np.ascontiguousarray(
            (kvec[sl].astype(np.float32) @ cell_inv.astype(np.float32)).T)
        km4 = np.concatenate([kmT, np.full((1, KSH), MAGIC, np.float32)],
                             axis=0)
        in_maps.append({
            "cts4": np.ascontiguousarray(cts4),
            "km4": np.ascontiguousarray(km4),
            "kmn": np.ascontiguousarray(-kmT),
            "qT": np.ascontiguousarray(q[b].reshape(NCH, 128).T,
                                       dtype=np.float32),
            "ef2": np.ascontiguousarray(np.concatenate([ef8, ef8], axis=1)),
            "ident": ident,
        })
    return in_maps


def _finalize(results, q, volume, bewald):
    recip = np.zeros((B, N), np.float32)
    for c in range(NCORES):
        b = c // (NCORES // B)
        recip[b] += results[c]["recp"].T.reshape(-1)
    scale1 = np.float32(BOHR / (math.pi * float(volume[0])))
    scale2 = np.float32(2.0 * float(bewald[0]) * BOHR / math.sqrt(math.pi))
    phi = (recip * scale1 - q.astype(np.float32) * scale2).astype(np.float32)
    e = (np.float32(0.5) * q.astype(np.float32) * phi).astype(np.float32)
    return e, phi


def kernel(coords, q, cell_inv, kvec, expfac, volume, bewald):
    from concourse.bass_utils import run_bass_kernel_spmd

    nc = _get_prog()
    in_maps = _make_in_maps(coords, q, cell_inv, kvec, expfac)
    res = run_bass_kernel_spmd(nc, in_maps, list(range(NCORES))).results
    return _finalize(res, q, volume, bewald)
